# revision 1
# baseline (speedup 1.0000x reference)
"""Causal self-attention (B=2, L=4096, D=768, H=12) on 8 TRN2 NeuronCores.

Sharding: core c -> batch b = c//4, head group g = c%4 (heads 3g..3g+2).
Per core: QKV projection for its 3 heads (bf16, Q pre-scaled on host),
causal flash-style attention with scores^T layout and row-packed matmul
pairs (two PE row-groups run concurrently), ones-column rowsums, chunked
multi-bank exp, partial output projection, and a 3-way split 4-core
add-ReduceScatter (earlier chunks overlap compute). Host reassembles the
full [2, 4096, 768] output and adds bo. bqkv/bo are zeros per the problem
spec; kernel() fails loudly if not.
"""

import sys

for _p in ("/opt/trn_rl_repo",):
    if _p not in sys.path:
        sys.path.insert(0, _p)

import numpy as np
import ml_dtypes

B, L, D, H = 2, 4096, 768, 12
Dh = D // H          # 64
HPC = 3              # heads per core
NCORES = 8
QB = 512             # q block (free dim of scores matmul)
KT = 128             # k tile (partition dim of scores^T)
NQ = L // QB         # 8
NKT = L // KT        # 32
KC = D // 128        # 6 contraction chunks for projections
CH = 3               # k-tiles per exp chunk (3 PSUM banks)

_CACHE = {}


def _build():
    import concourse.mybir as mybir
    import concourse.tile as tile
    from concourse import bacc

    bf16 = mybir.dt.bfloat16
    f32 = mybir.dt.float32
    Exp = mybir.ActivationFunctionType.Exp

    nc = bacc.Bacc("TRN2", target_bir_lowering=False, debug=False,
                   num_devices=NCORES)

    xT = nc.dram_tensor('xT', [D, L], bf16, kind='ExternalInput')
    wqk = nc.dram_tensor('wqk', [D, 384], bf16, kind='ExternalInput')
    wv = nc.dram_tensor('wv', [D, 192], bf16, kind='ExternalInput')
    wo = nc.dram_tensor('wo', [HPC * Dh, D], bf16, kind='ExternalInput')
    msk = nc.dram_tensor('msk', [KT, 4 * QB], bf16, kind='ExternalInput')
    out = nc.dram_tensor('out', [NQ * 128, D], bf16, kind='ExternalOutput')

    with tile.TileContext(nc) as tc:
        with tc.tile_pool(name='const', bufs=1) as cpool, \
             tc.tile_pool(name='work', bufs=3) as wpool, \
             tc.tile_pool(name='psum', bufs=1, space='PSUM') as pp, \
             tc.tile_pool(name='dram', bufs=1, space='DRAM') as dp:

            # ---------------- load phase ----------------
            xt = cpool.tile([128, KC, L], bf16)
            for kc in range(KC):
                nc.sync.dma_start(out=xt[:, kc, :], in_=xT[kc * 128:(kc + 1) * 128, :])
            wqk_sb = cpool.tile([128, KC, 384], bf16)
            wv_sb = cpool.tile([128, KC, 192], bf16)
            wo_sb = cpool.tile([64, HPC, D], bf16)
            for kc in range(KC):
                nc.sync.dma_start(out=wqk_sb[:, kc, :], in_=wqk[kc * 128:(kc + 1) * 128, :])
                nc.sync.dma_start(out=wv_sb[:, kc, :], in_=wv[kc * 128:(kc + 1) * 128, :])
            for h in range(HPC):
                nc.sync.dma_start(out=wo_sb[:, h, :], in_=wo[h * 64:(h + 1) * 64, :])
            msk_sb = cpool.tile([KT, 4 * QB], bf16)
            nc.sync.dma_start(out=msk_sb[:, :], in_=msk[:, :])
            ones = cpool.tile([128, 64], bf16)
            nc.vector.memset(ones[:, :], 1.0)

            # qkA: p0-63 = [q_h0 | k_h0], p64-127 = [q_h1 | k_h1]
            # qk2: p0-63 = [q_h2 | k_h2], p64-127 = duplicate
            qkA = cpool.tile([128, 2 * L], bf16)
            qk2 = cpool.tile([128, 2 * L], bf16)
            v_sb = cpool.tile([128, NKT, 195], bf16)
            for h in range(HPC):
                nc.vector.memset(v_sb[:, :, h * 65 + 64:h * 65 + 65], 1.0)
            yt = [cpool.tile([64, L], bf16, tag=f'yt{h}', name=f'yt{h}')
                  for h in range(HPC)]

            stags = ['sA', 'sB']

            # ---------------- QKV projection ----------------
            for n in range(NQ):
                tsl = slice(n * QB, (n + 1) * QB)
                for ct in range(3):
                    ps = pp.tile([128, CH * QB], f32, tag=stags[(3 * n + ct) % 2],
                                 bufs=1, name='ps_pj')
                    for kc in range(KC):
                        nc.tensor.matmul(ps[:, 0:QB],
                                         wqk_sb[:, kc, ct * 128:(ct + 1) * 128],
                                         xt[:, kc, tsl],
                                         start=(kc == 0), stop=(kc == KC - 1))
                    if ct == 0:
                        nc.vector.tensor_copy(qkA[:, tsl], ps[:, 0:QB])
                    elif ct == 1:
                        nc.vector.tensor_copy(qkA[:, L + n * QB:L + (n + 1) * QB],
                                              ps[:, 0:QB])
                    else:
                        nc.vector.tensor_copy(qk2[0:64, tsl], ps[0:64, 0:QB])
                        stq = wpool.tile([128, QB], bf16, tag='stq')
                        nc.vector.tensor_copy(stq[0:64, :], ps[0:64, 0:QB])
                        nc.sync.dma_start(out=qk2[64:128, tsl], in_=stq[0:64, :])
                        nc.vector.tensor_copy(qk2[64:128, L + n * QB:L + (n + 1) * QB],
                                              ps[64:128, 0:QB])
                        stk = wpool.tile([128, QB], bf16, tag='stk')
                        nc.vector.tensor_copy(stk[64:128, :], ps[64:128, 0:QB])
                        nc.sync.dma_start(out=qk2[0:64, L + n * QB:L + (n + 1) * QB],
                                          in_=stk[64:128, :])
            for m in range(NKT):
                psv = pp.tile([128, CH * QB], f32, tag=stags[m % 2], bufs=1,
                              name='ps_v')
                for kc in range(KC):
                    nc.tensor.matmul(psv[:, 0:192], xt[:, kc, m * 128:(m + 1) * 128],
                                     wv_sb[:, kc, :], start=(kc == 0),
                                     stop=(kc == KC - 1))
                for h in range(HPC):
                    nc.vector.tensor_copy(v_sb[:, m, h * 65:h * 65 + 64],
                                          psv[:, h * 64:(h + 1) * 64])

            # ---------------- attention ----------------
            def normalize(py, yt_t, qsl, stag):
                rs = wpool.tile([1, QB], bf16, tag='rs', name='rs')
                nc.vector.tensor_copy(rs[:, :], py[64:65, :])
                pb = pp.tile([128, CH * QB], f32, tag=stag, bufs=1, name='pb')
                nc.tensor.matmul(pb[0:64, 0:QB], ones[0:1, 0:64], rs[0:1, :],
                                 start=True, stop=True)
                rcp = wpool.tile([64, QB], f32, tag='rcp', name='rcp')
                nc.vector.reciprocal_approx_fast(out=rcp[:, :], in_=pb[0:64, 0:QB])
                nc.vector.tensor_mul(yt_t[:, qsl], py[0:64, :], rcp[:, :])

            def attn_pair(qA, kA, iA, ytA, hA, qB, kB, iB, ytB, hB):
                """Two causal-attention streams row-packed on the PE."""
                nkA, nkB = 4 * (iA + 1), 4 * (iB + 1)
                qslA = slice(iA * QB, (iA + 1) * QB)
                qslB = slice(iB * QB, (iB + 1) * QB)
                pyA = pp.tile([128, QB], f32, tag='yA', bufs=1, name='pyA')
                pyB = pp.tile([128, QB], f32, tag='yB', bufs=1, name='pyB')
                nch = (nkB + CH - 1) // CH
                first = {id(pyA): True, id(pyB): True}
                # diagonal chunks first: mask latency overlaps later chunks
                for c in list(range(nch))[::-1]:
                    nt = min(CH, nkB - c * CH)
                    nA = max(0, min(CH, nkA - c * CH))
                    sA = (pp.tile([128, CH * QB], f32, tag='sA', bufs=1, name='sA')
                          if nA else None)
                    sB = pp.tile([128, CH * QB], f32, tag='sB', bufs=1, name='sB')
                    for t in range(nt):
                        kb = c * CH + t
                        if t < nA:
                            nc.tensor.matmul(sA[:, t * QB:(t + 1) * QB],
                                             kA[:, kb * KT:(kb + 1) * KT],
                                             qA[:, qslA], start=True, stop=True)
                        nc.tensor.matmul(sB[:, t * QB:(t + 1) * QB],
                                         kB[:, kb * KT:(kb + 1) * KT],
                                         qB[:, qslB], start=True, stop=True)
                    ptA = (wpool.tile([128, CH * QB], bf16, tag='pt', bufs=8,
                                      name='ptA') if nA else None)
                    ptB = wpool.tile([128, CH * QB], bf16, tag='pt', bufs=8,
                                     name='ptB')
                    if nA:
                        nc.scalar.activation(ptA[:, 0:nA * QB], sA[:, 0:nA * QB], Exp)
                    nc.scalar.activation(ptB[:, 0:nt * QB], sB[:, 0:nt * QB], Exp)
                    for t in range(nA):
                        r = c * CH + t - 4 * iA
                        if r >= 0:
                            nc.vector.tensor_mul(ptA[:, t * QB:(t + 1) * QB],
                                                 ptA[:, t * QB:(t + 1) * QB],
                                                 msk_sb[:, r * QB:(r + 1) * QB])
                    for t in range(nt):
                        r = c * CH + t - 4 * iB
                        if r >= 0:
                            nc.vector.tensor_mul(ptB[:, t * QB:(t + 1) * QB],
                                                 ptB[:, t * QB:(t + 1) * QB],
                                                 msk_sb[:, r * QB:(r + 1) * QB])
                    for t in range(nA):
                        kb = c * CH + t
                        nc.tensor.matmul(pyA[0:65, :],
                                         v_sb[:, kb, hA * 65:hA * 65 + 65],
                                         ptA[:, t * QB:(t + 1) * QB],
                                         start=first[id(pyA)] and t == 0,
                                         stop=(c == 0 and t == nA - 1))
                        if t == 0:
                            first[id(pyA)] = False
                    for t in range(nt):
                        kb = c * CH + t
                        nc.tensor.matmul(pyB[0:65, :],
                                         v_sb[:, kb, hB * 65:hB * 65 + 65],
                                         ptB[:, t * QB:(t + 1) * QB],
                                         start=first[id(pyB)] and t == 0,
                                         stop=(c == 0 and t == nt - 1))
                        if t == 0:
                            first[id(pyB)] = False
                normalize(pyA, ytA, qslA, 'sA')
                normalize(pyB, ytB, qslB, 'sB')

            # ---------- attention + partial outproj + split reduce ----------
            pch1 = dp.tile([4 * QB, D], bf16, name='pch1')
            rsch1 = dp.tile([4 * 128, D], bf16, name='rsch1')
            pch2 = dp.tile([2 * QB, D], bf16, name='pch2')
            rsch2 = dp.tile([2 * 128, D], bf16, name='rsch2')
            pch3 = dp.tile([2 * QB, D], bf16, name='pch3')
            rsch3 = dp.tile([2 * 128, D], bf16, name='rsch3')
            groups = [[0, 1, 2, 3], [4, 5, 6, 7]]

            for p in range(4):
                for i in (2 * p, 2 * p + 1):
                    attn_pair(qkA[0:64, 0:L], qkA[0:64, L:2 * L], i, yt[0], 0,
                              qkA[64:128, 0:L], qkA[64:128, L:2 * L], i, yt[1], 1)
                attn_pair(qk2[0:64, 0:L], qk2[0:64, L:2 * L], 2 * p, yt[2], 2,
                          qk2[64:128, 0:L], qk2[64:128, L:2 * L], 2 * p + 1,
                          yt[2], 2)
                pch = pch1 if p < 2 else (pch2 if p == 2 else pch3)
                base = 0 if p < 2 else 2 * p * QB
                for i in (2 * p, 2 * p + 1):
                    for mm in range(QB // 128):
                        tok = i * QB + mm * 128
                        ot = wpool.tile([128, D], bf16, tag='ot', name='ot')
                        for dj, (d0, dw) in enumerate(((0, QB), (QB, 256))):
                            po = pp.tile([128, QB], f32,
                                         tag=('yA', 'yB')[(mm + dj) % 2], bufs=1,
                                         name='po')
                            for h in range(HPC):
                                nc.tensor.matmul(po[:, 0:dw],
                                                 yt[h][:, tok:tok + 128],
                                                 wo_sb[:, h, d0:d0 + dw],
                                                 start=(h == 0),
                                                 stop=(h == HPC - 1))
                            nc.vector.tensor_copy(ot[:, d0:d0 + dw], po[:, 0:dw])
                        row = i * QB + mm * 128 - base
                        nc.sync.dma_start(out=pch[row:row + 128, :], in_=ot[:, :])
                if p == 1:
                    nc.gpsimd.collective_compute(
                        "ReduceScatter", mybir.AluOpType.add,
                        replica_groups=groups, ins=[pch1.opt()], outs=[rsch1.opt()])
                if p == 2:
                    nc.gpsimd.collective_compute(
                        "ReduceScatter", mybir.AluOpType.add,
                        replica_groups=groups, ins=[pch2.opt()], outs=[rsch2.opt()])
                if p == 3:
                    nc.gpsimd.collective_compute(
                        "ReduceScatter", mybir.AluOpType.add,
                        replica_groups=groups, ins=[pch3.opt()], outs=[rsch3.opt()])
            nc.sync.dma_start(out=out[0:512, :], in_=rsch1[:, :])
            nc.sync.dma_start(out=out[512:768, :], in_=rsch2[:, :])
            nc.sync.dma_start(out=out[768:1024, :], in_=rsch3[:, :])
    nc.compile()
    return nc


def kernel(x, Wqkv, bqkv, Wo, bo):
    from concourse.bass_utils import run_bass_kernel_spmd

    if 'nc' not in _CACHE:
        _CACHE['nc'] = _build()
    nc = _CACHE['nc']

    bf = ml_dtypes.bfloat16
    x = np.asarray(x, np.float32)
    Wqkv = np.asarray(Wqkv, np.float32)
    bqkv = np.asarray(bqkv, np.float32)
    Wo = np.asarray(Wo, np.float32)
    bo = np.asarray(bo, np.float32)

    # device graph omits the qkv bias adds (always zeros per problem spec)
    assert np.abs(bqkv).max() == 0.0, "nonzero bqkv unsupported by this kernel"

    scale = 1.0 / np.sqrt(Dh)
    Q, K, V = Wqkv[:, 0:D], Wqkv[:, D:2 * D], Wqkv[:, 2 * D:3 * D]

    kl = np.arange(KT)[:, None]
    ql = np.arange(QB)[None, :]
    msk = np.ascontiguousarray(
        np.concatenate([(ql >= kl + KT * r) for r in range(4)], axis=1)
    ).astype(bf)

    in_maps = []
    for c in range(NCORES):
        b, g = divmod(c, 4)
        hs = [3 * g, 3 * g + 1, 3 * g + 2]
        cols = lambda W, h: W[:, h * Dh:(h + 1) * Dh]
        wqk_np = np.concatenate(
            [cols(Q, hs[0]) * scale, cols(Q, hs[1]) * scale,
             cols(K, hs[0]), cols(K, hs[1]),
             cols(Q, hs[2]) * scale, cols(K, hs[2])], axis=1).astype(bf)
        wv_np = np.concatenate([cols(V, h) for h in hs], axis=1).astype(bf)
        wo_np = Wo[3 * g * Dh:(3 * g + 3) * Dh, :].astype(bf)
        xT_np = np.ascontiguousarray(x[b].T).astype(bf)
        in_maps.append({
            'xT': xT_np, 'wqk': np.ascontiguousarray(wqk_np),
            'wv': np.ascontiguousarray(wv_np),
            'wo': np.ascontiguousarray(wo_np),
            'msk': msk,
        })

    res = run_bass_kernel_spmd(nc, in_maps, core_ids=list(range(NCORES)))

    out = np.empty((B, L, D), np.float32)
    for c in range(NCORES):
        b, g = divmod(c, 4)
        o = res.results[c]['out'].astype(np.float32)
        out[b, g * QB:(g + 1) * QB, :] = o[0:512, :]
        out[b, 2048 + g * 256:2048 + (g + 1) * 256, :] = o[512:768, :]
        out[b, 3072 + g * 256:3072 + (g + 1) * 256, :] = o[768:1024, :]
    out += bo[None, None, :]
    return out

